# revision 3
# baseline (speedup 1.0000x reference)
"""NCC loss (local normalized cross-correlation, window 9^3) on 8 Trainium2
NeuronCores — v3.

Reference: 5 channels [I, J, I^2, J^2, IJ] box-filtered (separable 9-tap mean,
SAME zero-pad) over a 192^3 volume; cc = sigma12^2/(sigma1^2*sigma2^2+eps);
output = 1 - mean(cc).

Sharding: depth axis. Core c computes output slices [24c, 24c+24), reading
padded input slices [24c, 24c+32) of the (+4 both ends) padded volume.

v3 changes vs v2 (217us):
  - fp16 everywhere off-PSUM (better mantissa than bf16, same op rates);
    inputs are centered (x-0.5) ON THE HOST, pad slots = -0.5 (exact for all
    stats since every window's support is inside the ext slab).
  - prep is 2 ops/pair/rowgroup: ACT Square -> [I^2 J^2], DVE mult -> IJ.
    I, J channels feed the H-matmul directly from the input tile (no copy).
  - the D-diff (hi-lo of cumsum snapshots) is FOLDED INTO the W-matmul:
    snapshots are drained per-slice into padded 128-col (wc, ch) blocks,
    DMA-x-bar transposed per z-PAIR (32 transposes), and the W-pass runs
    8 accumulating matmuls per out-slice: +band @ tt[oz+8], -band @ tt[oz-1].
    This removes all 96 diff TTs and their tiles/memsets from DVE.
  - cc stage batched over 2 out-slices (halves per-op overhead); squares and
    den products moved to the (idle) GPSIMD engine; drains split ACT/DVE
    (A-snapshots+wc1-ff on ACT, B-snapshots+wc0-ff on DVE).
  - fp16 magic+Newton reciprocal (MAGIC 0x7798), max-guard at fp16 min-normal.

Numerically validated in numpy (val3.py): rel err ~2.3e-4 vs f64 reference.
"""

import sys

import numpy as np

sys.path.insert(0, "/opt/trn_rl_repo")

import contextlib

import concourse.bacc as bacc
import concourse.mybir as mybir
from concourse import tile
from concourse.bass_utils import run_bass_kernel_spmd

F32 = mybir.dt.float32
FP16 = mybir.dt.float16
I16 = mybir.dt.int16
AOT = mybir.AluOpType
ACTF = mybir.ActivationFunctionType
AXL = mybir.AxisListType

H = 192
W = 192
D_TOT = 192
HE = 200   # extended h (4 pad each side)
WE = 200   # extended w
PAD = 4
N_CORES = 8

HA = 112   # H-pass out: ext rows 4..115  == orig h 0..111
HB = 80    # H-pass out: ext rows 116..195 == orig h 112..191
KT = 128   # chanT partitions: ext-h 0..127
KB = 88    # chanB partitions: ext-h 112..199

BAND_C = 1.0 / 27.0
NCH = 5
CH = 8     # slices per input-load chunk
ZG = 2     # slices per transpose group

EPS16 = 6.103515625e-05   # fp16 min normal; den guard (den >= ~2e-3 for this data)
MAGIC = 0x7798            # fp16 reciprocal seed: bits(r0) = MAGIC - bits(x)


def _band(rows, cols, val):
    k = np.arange(rows)[:, None]
    m = np.arange(cols)[None, :]
    return np.where((k - m >= 0) & (k - m <= 8), val, 0.0).astype(np.float32)


def make_consts():
    # [120, 208]: cols 0:112 = +H-band (120x112); cols 112:208 = -W-band
    # (104x96, rows 104:120 zero).  The +W-band is band[0:104, 0:96].
    b = np.zeros((120, 208), np.float32)
    b[:, 0:112] = _band(120, 112, BAND_C)
    b[0:104, 112:208] = -_band(104, 96, BAND_C)
    return b.astype(np.float16)


def build_program(din, dout):
    assert din == dout + 2 * PAD
    nc = bacc.Bacc(
        "TRN2", target_bir_lowering=False, debug=False, num_devices=N_CORES
    )

    xin_d = nc.dram_tensor("xin", [din, HE, 2 * WE], FP16, kind="ExternalInput")
    band_d = nc.dram_tensor("band", [120, 208], FP16, kind="ExternalInput")
    out_d = nc.dram_tensor("out", [96, 1], F32, kind="ExternalOutput")

    xin = xin_d.ap()
    NB = 2 * NCH            # 10 transpose blocks per slice
    TTF = ZG * NB * H       # tt free size (3840)
    SNF = ZG * NB * 128     # snap free size (2560)

    with tile.TileContext(nc) as tc, contextlib.ExitStack() as ctx:
        consts = ctx.enter_context(tc.tile_pool(name="consts", bufs=1))
        xts = ctx.enter_context(tc.tile_pool(name="xts", bufs=1))
        chans = ctx.enter_context(tc.tile_pool(name="chans", bufs=3))
        snaps = ctx.enter_context(tc.tile_pool(name="snaps", bufs=1))
        tts = ctx.enter_context(tc.tile_pool(name="tts", bufs=1))
        ffs = ctx.enter_context(tc.tile_pool(name="ffs", bufs=2))
        ccs = ctx.enter_context(tc.tile_pool(name="ccs", bufs=2))
        accp = ctx.enter_context(tc.tile_pool(name="accp", bufs=1))
        ps_h = ctx.enter_context(tc.tile_pool(name="psh", bufs=1, space="PSUM"))
        ps_w = ctx.enter_context(tc.tile_pool(name="psw", bufs=1, space="PSUM"))

        band = consts.tile([120, 208], FP16, tag="band")
        nc.sync.dma_start(band[:], band_d.ap())

        acc = accp.tile([96, dout // 2], F32, tag="acc")
        nc.vector.memset(acc[:], 0.0)

        # input chunk tiles, ping-pong
        xtT = [xts.tile([KT, CH * 2 * WE], FP16, tag=f"xtT{p}", name=f"xtT{p}")
               for p in range(2)]
        xtB = [xts.tile([KB, CH * 2 * WE], FP16, tag=f"xtB{p}", name=f"xtB{p}")
               for p in range(2)]

        def load_chunk(c):
            z0 = c * CH
            src = xin[z0:z0 + CH, :, :]
            xt3 = xtT[c % 2].rearrange("p (z w) -> p z w", z=CH)
            xb3 = xtB[c % 2].rearrange("p (z w) -> p z w", z=CH)
            nc.sync.dma_start(xt3[:, :, :], src[:, 0:KT, :].rearrange("z h w -> h z w"))
            nc.sync.dma_start(xb3[:, :, :], src[:, HE - KB:HE, :].rearrange("z h w -> h z w"))

        # snapshot tiles (padded (zi, wc, ch, 128) blocks), ping-pong per group
        snapA = [snaps.tile([HA, SNF], FP16, tag=f"snapA{p}", name=f"snapA{p}")
                 for p in range(2)]
        snapB = [snaps.tile([HB, SNF], FP16, tag=f"snapB{p}", name=f"snapB{p}")
                 for p in range(2)]
        for t in (*snapA, *snapB):
            nc.vector.memset(t[:], 0.0)

        # transposed-snapshot tiles, ring of 6
        NTT = 6
        tt_t = [tts.tile([128, TTF], FP16, tag=f"tt{i}", name=f"tt{i}")
                for i in range(NTT)]

        # PSUM: psA+psB = 4 banks (H cumsum), pw = 4 banks (W out) -> 8 total
        psA = ps_h.tile([HA, 1024], F32, tag="psA")
        psB = ps_h.tile([HB, 1024], F32, tag="psB")
        pw = ps_w.tile([96, 2048], F32, tag="pw")
        pw3 = pw.rearrange("p (b w) -> p b w", b=2)   # [96, 2wc, 1024]

        def prep_pair(z0):
            c = (z0 // CH) % 2
            zi0 = z0 % CH
            chanT = chans.tile([KT, 2 * 600], FP16, tag="chanT", name="chanT")
            chanB = chans.tile([KB, 2 * 600], FP16, tag="chanB", name="chanB")
            for ch_t, xt in ((chanT, xtT[c]), (chanB, xtB[c])):
                np_ = ch_t.shape[0]
                x3 = xt.rearrange("p (z w) -> p z w", z=CH)
                raw = x3[0:np_, zi0:zi0 + 2, :]          # [np, 2, 400]
                c3 = ch_t.rearrange("p (z f) -> p z f", z=2)
                # [I^2, J^2] on ACT
                nc.scalar.activation(c3[:, :, 0:400], raw, ACTF.Square)
                # IJ on DVE
                nc.vector.tensor_tensor(
                    c3[:, :, 400:600], raw[:, :, 0:WE], raw[:, :, WE:2 * WE],
                    AOT.mult,
                )
            return chanT, chanB

        def h_pass(z, chanT, chanB, zi):
            start = z == 0
            c = (z // CH) % 2
            zz = z % CH
            # 3 matmuls per rowgroup; PSUM channel layout (stride 200):
            #   I^2@0, J^2@200, IJ@400, I@600, J@800
            for ps, ch_t, xt, ncon, nout in (
                (psA, chanT, xtT[c], 120, HA),
                (psB, chanB, xtB[c], KB, HB),
            ):
                b = band[0:ncon, 0:nout]
                c2 = ch_t.rearrange("p (z f) -> p z f", z=2)[0:ncon, zi, :]
                x2 = xt.rearrange("p (z w) -> p z w", z=CH)[0:ncon, zz, :]
                nc.tensor.matmul(ps[:, 0:512], b, c2[0:ncon, 0:512],
                                 start=start, stop=True, skip_group_check=True)
                nc.tensor.matmul(ps[:, 512:600], b, c2[0:ncon, 512:600],
                                 start=start, stop=True, skip_group_check=True)
                nc.tensor.matmul(ps[:, 600:1000], b, x2[0:ncon, 0:400],
                                 start=start, stop=True, skip_group_check=True)

            # snapshot drains into padded (wc, ch, 128) blocks.
            # A rowgroup -> ACT, B rowgroup -> DVE.
            pp = (z // ZG) % 2
            zi2 = z % ZG
            for ps, sn, eng_copy in (
                (psA, snapA[pp], nc.scalar.copy),
                (psB, snapB[pp], lambda o, i: nc.vector.tensor_copy(o, i)),
            ):
                psv = ps[:, 0:1000].rearrange("p (c w) -> p c w", c=NCH)
                s5 = sn.rearrange("p (z b c w) -> p z b c w", z=ZG, b=2, c=NCH)
                eng_copy(s5[:, zi2, 0, :, 0:104], psv[:, :, 0:104])
                eng_copy(s5[:, zi2, 1, :, 0:104], psv[:, :, 96:200])

        def transpose_group(g):
            # snap pair (z = 2g, 2g+1) -> tt ring buffer [128, (zi b c), H]
            tt = tt_t[g % NTT]
            t3 = tt.rearrange("p (n h) -> p n h", h=H)     # [128, 20, 192]
            nc.sync.dma_start_transpose(t3[:, :, 0:HA], snapA[g % 2][:])
            nc.sync.dma_start_transpose(t3[:, :, HA:H], snapB[g % 2][:])

        def w_pass(oz):
            zhi = oz + 8
            hi = tt_t[(zhi // ZG) % NTT]
            hi_base = (zhi % ZG) * NB * H
            bw_hi = band[0:104, 0:96]
            mms = [(hi, hi_base, bw_hi, True)]
            if oz > 0:
                zlo = oz - 1
                lo = tt_t[(zlo // ZG) % NTT]
                lo_base = (zlo % ZG) * NB * H
                bw_lo = band[0:104, 112:208]
                mms.append((lo, lo_base, bw_lo, False))
            nphase = len(mms)
            for pi, (tt, base, bw, first) in enumerate(mms):
                last = pi == nphase - 1
                for wc in range(2):
                    for p in range(2):
                        wd = 512 if p == 0 else 448
                        c0 = base + wc * NCH * H + p * 512
                        nc.tensor.matmul(
                            pw3[:, wc, p * 512:p * 512 + wd],
                            bw, tt[0:104, c0:c0 + wd],
                            start=first, stop=last,
                        )

        def ff_drain(oz, ff):
            ozp = oz % 2
            f4 = ff.rearrange("p (o b w) -> p o b w", o=2, b=2)
            nc.vector.tensor_copy(f4[:, ozp, 0, :], pw3[:, 0, 0:960])
            nc.scalar.copy(f4[:, ozp, 1, :], pw3[:, 1, 0:960])

        def cc_batch(b, ff):
            # views [96, 2oz, 2wc, X]; ff channel layout per wc:
            #   I^2@0, J^2@192, IJ@384, I@576, J@768
            f4 = ff.rearrange("p (o b w) -> p o b w", o=2, b=2)
            F_CONV = f4[:, :, :, 0:2 * H]
            F_IJ = f4[:, :, :, 2 * H:3 * H]
            F_I = f4[:, :, :, 3 * H:4 * H]
            F_J = f4[:, :, :, 4 * H:5 * H]
            F_SQ = f4[:, :, :, 3 * H:5 * H]

            sc = ccs.tile([96, 2 * 2 * 1152], FP16, tag="sc", name="sc")
            s4 = sc.rearrange("p (o b w) -> p o b w", o=2, b=2)
            t1 = s4[:, :, :, 0:H]
            s12 = s4[:, :, :, H:2 * H]
            sqs = s4[:, :, :, 2 * H:4 * H]
            sg = s4[:, :, :, 4 * H:6 * H]
            sg1 = s4[:, :, :, 4 * H:5 * H]
            sg2 = s4[:, :, :, 5 * H:6 * H]
            scd = ccs.tile([96, 2 * 2 * 768], FP16, tag="scd", name="scd")
            d4 = scd.rearrange("p (o b w) -> p o b w", o=2, b=2)
            den = d4[:, :, :, 0:H]
            r0 = d4[:, :, :, H:2 * H]
            tq = d4[:, :, :, 2 * H:3 * H]
            r1n = d4[:, :, :, 3 * H:4 * H]
            den2 = t1                     # t1 dead after s12
            s2f = s4[:, :, :, 2 * H:3 * H]  # sqs dead after sg

            nc.gpsimd.tensor_tensor(sqs, F_SQ, F_SQ, AOT.mult)
            nc.vector.tensor_tensor(t1, F_I, F_J, AOT.mult)
            nc.vector.tensor_tensor(s12, F_IJ, t1, AOT.subtract)
            nc.vector.tensor_tensor(sg, F_CONV, sqs, AOT.subtract)
            nc.gpsimd.tensor_tensor(den, sg1, sg2, AOT.mult)
            nc.vector.tensor_scalar_max(den2, den, EPS16)
            # reciprocal seed: bits(r0) = MAGIC - bits(den2)
            nc.vector.tensor_scalar(
                r0.bitcast(I16), den2.bitcast(I16), -1, MAGIC,
                AOT.mult, AOT.add,
            )
            # one Newton step, sign-folded: r1n = (den2*r0 - 2)*r0 = -recip
            nc.vector.tensor_tensor(tq, den2, r0, AOT.mult)
            nc.vector.scalar_tensor_tensor(
                r1n, tq, 2.0, r0, AOT.subtract, AOT.mult
            )
            nc.vector.tensor_tensor(s2f, s12, s12, AOT.mult)
            # cc = (-s2f) * r1n = s12^2 * recip(den), accumulated into acc
            nc.vector.scalar_tensor_tensor(
                tq, s2f, -1.0, r1n, AOT.mult, AOT.mult,
                accum_out=acc[:, b:b + 1],
            )

        ff_cur = [None]
        for z0 in range(0, din, 2):
            if z0 == 0:
                load_chunk(0)
                load_chunk(1)
            elif z0 % CH == 0 and z0 // CH + 1 < din // CH:
                load_chunk(z0 // CH + 1)
            chanT, chanB = prep_pair(z0)
            for zi in range(2):
                h_pass(z0 + zi, chanT, chanB, zi)
            g = z0 // ZG
            transpose_group(g)
            for oz in (z0 - 8, z0 - 7):
                if 0 <= oz < dout:
                    if oz % 2 == 0:
                        ff_cur[0] = ffs.tile([96, 2 * 2 * 960], FP16,
                                             tag="ff", name="ff")
                    w_pass(oz)
                    ff_drain(oz, ff_cur[0])
                    if oz % 2 == 1:
                        cc_batch(oz // 2, ff_cur[0])

        accv = accp.tile([96, 1], F32, tag="accv")
        nc.vector.tensor_reduce(accv[:], acc[:], AXL.X, AOT.add)
        nc.sync.dma_start(out_d.ap(), accv[:])

    nc.compile()
    return nc


_PROGRAM_CACHE = {}


def _get_program(din, dout):
    key = (din, dout)
    if key not in _PROGRAM_CACHE:
        _PROGRAM_CACHE[key] = build_program(din, dout)
    return _PROGRAM_CACHE[key]


def make_in_maps(pred, target):
    pred = np.asarray(pred).reshape(D_TOT, H, W).astype(np.float32)
    targ = np.asarray(target).reshape(D_TOT, H, W).astype(np.float32)

    dout = D_TOT // N_CORES
    din = dout + 2 * PAD

    # one interleaved, padded, centered fp16 volume: [D+8, 200, 400], pad=-0.5
    big = np.full((D_TOT + 2 * PAD, HE, 2 * WE), -0.5, np.float16)
    big[PAD:-PAD, PAD:PAD + H, PAD:PAD + W] = (targ - 0.5).astype(np.float16)
    big[PAD:-PAD, PAD:PAD + H, WE + PAD:WE + PAD + W] = (pred - 0.5).astype(np.float16)

    band = make_consts()
    in_maps = []
    for c in range(N_CORES):
        s = c * dout
        in_maps.append(
            {
                "xin": np.ascontiguousarray(big[s:s + din]),
                "band": band,
            }
        )
    return in_maps, din, dout


def kernel(pred, target):
    in_maps, din, dout = make_in_maps(pred, target)
    nc = _get_program(din, dout)
    res = run_bass_kernel_spmd(nc, in_maps, core_ids=list(range(N_CORES)))
    total = sum(float(r["out"].astype(np.float64).sum()) for r in res.results)
    return np.float32(1.0 - total / float(D_TOT * H * W))
